# revision 1
# baseline (speedup 1.0000x reference)
"""Bidirectional GQA attention block (B=4,T=2048,C=2048,H=16,KVH=4) on 8 TRN2 cores.

Sharding: data-parallel over (batch, seq-half): core c handles batch b=c//2,
query tokens [r0, r0+1024) with r0=(c%2)*1024.  k/v are computed for the full
batch sequence on each core (2x duplicated work, ~8% overhead) so no cross-core
communication is needed; the final output is a pure concatenation.

Per-core pipeline (all matmuls in float32r = full-rate fp32 on the PE array):
  P1: q^T = (wq^T x^T) channel-major, k^T channel-major, v token-major.
      Sum-of-squares for RMSNorm via ones-matmul (partition-axis reduce).
      q^T,k^T,v staged to DRAM scratch.
  P2: RMSNorm scale + RoPE folded into per-token cos/sin tables
      (q tables also pre-scaled by 1/sqrt(head_dim)); rope as
      qA*c2 + qSwap*s2 where qSwap is a swapped-half DMA re-read.
      logits^T = k_h q_h^T per head, exp on ACT, denominator via ones-matmul,
      y^T = v^T S accumulated in PSUM, divided by denominator.
  P3: out = y^T.T wo with PSUM accumulation over the 16 head-chunks.
"""
import sys
import os

sys.path.insert(0, "/opt/trn_rl_repo")

import numpy as np

B, T, C = 4, 2048, 2048
N_HEAD, N_KV_HEAD = 16, 4
HEAD_DIM = C // N_HEAD  # 128
KV_DIM = N_KV_HEAD * HEAD_DIM  # 512
EPS = 1e-5
TQ = 1024  # query tokens per core
N_CORES = 8

_CACHE = {}


def _build_nc(reps=1, trace_sim=False):
    import concourse.bass as bass
    import concourse.mybir as mybir
    import concourse.tile as tile
    from concourse import bacc

    F32 = mybir.dt.float32
    F32R = mybir.dt.float32r
    AF = mybir.ActivationFunctionType

    nc = bacc.Bacc("TRN2", target_bir_lowering=False, debug=False)

    def ein(name, shape):
        return nc.dram_tensor(name, shape, F32, kind="ExternalInput").ap()

    xT = ein("xT", [C, T])          # x[b].T  (c_in, tok)
    xTq = ein("xTq", [C, TQ])       # x[b].T[:, r0:r0+TQ]
    wq = ein("wq", [C, C])
    wk = ein("wk", [C, KV_DIM])
    wv = ein("wv", [C, KV_DIM])
    wo = ein("wo", [C, C])
    c2q = ein("c2q", [128, TQ])     # [cos;cos] / sqrt(HEAD_DIM), q token slice
    s2q = ein("s2q", [128, TQ])     # [sin;-sin] / sqrt(HEAD_DIM)
    c2k = ein("c2k", [128, T])
    s2k = ein("s2k", [128, T])
    qnw = ein("qnw", [128, 16])     # q_norm_w.reshape(16,128).T
    knw = ein("knw", [128, 4])
    out = nc.dram_tensor("out", [TQ, C], F32, kind="ExternalOutput").ap()

    ones_d = nc.inline_tensor(np.ones((128, 1), np.float32), name="onesc").ap()
    onesq_d = nc.inline_tensor(
        np.full((128, 1), 1.0 / C, np.float32), name="onesqc"
    ).ap()
    onesk_d = nc.inline_tensor(
        np.full((128, 1), 1.0 / KV_DIM, np.float32), name="oneskc"
    ).ap()
    eps_d = nc.inline_tensor(np.full((1, 1), EPS, np.float32), name="epsc").ap()

    # DRAM scratch
    qTs = nc.dram_tensor("qTs", [C, TQ], F32).ap()        # q^T * w (pre rope/rs)
    kTs = nc.dram_tensor("kTs", [KV_DIM, T], F32).ap()
    vs = nc.dram_tensor("vs", [T, KV_DIM], F32R).ap()     # token-major v
    yTs = nc.dram_tensor("yTs", [C, TQ], F32R).ap()       # y^T

    def r3(ap, p=128):
        # (c*p, n) -> (c, p, n)
        return ap.rearrange("(c p) n -> c p n", p=p)

    def rp(ap, p=128):
        # (c*p, n) -> (p, c, n)
        return ap.rearrange("(c p) n -> p c n", p=p)

    with tile.TileContext(nc, trace_sim=trace_sim) as tc:
        with tc.tile_pool(name="const", bufs=1) as cpool:
            ones_t = cpool.tile([128, 1], F32R, name="ones_t")
            nc.sync.dma_start(ones_t[:], ones_d.bitcast(F32R))
            onesq_t = cpool.tile([128, 1], F32R, name="onesq_t")
            nc.sync.dma_start(onesq_t[:], onesq_d.bitcast(F32R))
            onesk_t = cpool.tile([128, 1], F32R, name="onesk_t")
            nc.sync.dma_start(onesk_t[:], onesk_d.bitcast(F32R))
            eps_t = cpool.tile([1, 1], F32, name="eps_t")
            nc.sync.dma_start(eps_t[:], eps_d)
            qnw_t = cpool.tile([128, 16], F32, name="qnw_t")
            nc.sync.dma_start(qnw_t[:], qnw)
            knw_t = cpool.tile([128, 4], F32, name="knw_t")
            nc.sync.dma_start(knw_t[:], knw)
            rs_q = cpool.tile([1, TQ], F32, name="rs_q")
            rs_k = cpool.tile([1, T], F32, name="rs_k")

            for rep in range(reps):
                # ---------------- rope tables (loaded early, scaled in place) ----------------
                with tc.tile_pool(name="tabs", bufs=1) as ptab:
                    c2qs = ptab.tile([128, TQ], F32, name="c2qs")
                    nc.sync.dma_start(c2qs[:], c2q)
                    s2qs = ptab.tile([128, TQ], F32, name="s2qs")
                    nc.sync.dma_start(s2qs[:], s2q)
                    c2ks = ptab.tile([128, T], F32, name="c2ks")
                    nc.sync.dma_start(c2ks[:], c2k)
                    s2ks = ptab.tile([128, T], F32, name="s2ks")
                    nc.sync.dma_start(s2ks[:], s2k)
                    pwk = tc.alloc_tile_pool(name="wktp", bufs=1)
                    wkt = pwk.tile([128, 16, KV_DIM], F32R, name="wkt")
                    for kc4 in range(4):
                        sl = slice(kc4 * 4, kc4 * 4 + 4)
                        nc.sync.dma_start(wkt[:, sl, :], rp(wk)[:, sl, :].bitcast(F32R))
                    # ---------------- P1a: q^T projection ----------------
                    with tc.tile_pool(name="p1q", bufs=1) as p1, \
                         tc.tile_pool(name="wqlp", bufs=2) as pw, \
                         tc.tile_pool(name="ev1", bufs=2) as pe, \
                         tc.tile_pool(name="tmp1", bufs=2) as pt, \
                         tc.tile_pool(name="pp1", bufs=4, space="PSUM") as pp, \
                         tc.tile_pool(name="ssqp", bufs=1, space="PSUM") as pps:
                        xqs = []
                        for tq in range(2):
                            xq = p1.tile([128, 16, 512], F32R, name=f"xq{tq}",
                                         tag=f"xq{tq}")
                            for kc in range(16):
                                nc.sync.dma_start(
                                    xq[:, kc, :],
                                    rp(xTq)[:, kc, tq * 512:(tq + 1) * 512].bitcast(F32R),
                                )
                            xqs.append(xq)
                        ssq_ps = [
                            pps.tile([1, 512], F32, name=f"ssqq{tq}", tag=f"ssqq{tq}")
                            for tq in range(2)
                        ]
                        for cout in range(16):
                            wql = pw.tile([128, 16, 128], F32R, name="wql", tag="wql")
                            nc.sync.dma_start(
                                wql[:],
                                rp(wq)[:, :, cout * 128:(cout + 1) * 128].bitcast(F32R),
                            )
                            for tq in range(2):
                                ps = pp.tile([128, 512], F32, name="psq", tag="ps")
                                for kc in range(16):
                                    nc.tensor.matmul(
                                        ps[:],
                                        wql[:, kc, :],
                                        xqs[tq][:, kc, :],
                                        start=(kc == 0),
                                        stop=(kc == 15),
                                    )
                                qsb = pe.tile([128, 512], F32, name="qsb", tag="qsb")
                                nc.scalar.activation(
                                    qsb[:], ps[:], AF.Copy, scale=qnw_t[:, cout:cout + 1]
                                )
                                nc.sync.dma_start(
                                    r3(qTs)[cout, :, tq * 512:(tq + 1) * 512], qsb[:]
                                )
                                sq = pt.tile([128, 512], F32R, name="sqq", tag="sq")
                                nc.scalar.activation(sq[:], ps[:], AF.Square)
                                nc.tensor.matmul(
                                    ssq_ps[tq][:],
                                    onesq_t[:],
                                    sq[:],
                                    start=(cout == 0),
                                    stop=(cout == 15),
                                )
                        for tq in range(2):
                            sd = pe.tile([1, 512], F32, name="sdq", tag="sdq")
                            nc.scalar.activation(
                                sd[:], ssq_ps[tq][:], AF.Sqrt, bias=eps_t[:]
                            )
                            nc.vector.reciprocal(rs_q[:, tq * 512:(tq + 1) * 512], sd[:])

                    # ---------------- P1b: k^T and v projections ----------------
                    with tc.tile_pool(name="wkv", bufs=1) as pwkv, \
                         tc.tile_pool(name="xsp", bufs=2) as pxs, \
                         tc.tile_pool(name="ev2", bufs=2) as pe, \
                         tc.tile_pool(name="tmp2", bufs=2) as pt, \
                         tc.tile_pool(name="pp2", bufs=4, space="PSUM") as pp, \
                         tc.tile_pool(name="ssqk", bufs=2, space="PSUM") as pps:
                        wvt = pwkv.tile([128, 16, KV_DIM], F32R, name="wvt")
                        for kc4 in range(4):
                            sl = slice(kc4 * 4, kc4 * 4 + 4)
                            nc.sync.dma_start(wvt[:, sl, :], rp(wv)[:, sl, :].bitcast(F32R))
                        for tk in range(4):
                            xs = []
                            for kc in range(16):
                                xc = pxs.tile([128, 512], F32R, name=f"xsc{kc}",
                                              tag="xsc", bufs=20)
                                nc.sync.dma_start(
                                    xc[:],
                                    rp(xT)[:, kc, tk * 512:(tk + 1) * 512].bitcast(F32R),
                                )
                                xs.append(xc)
                            ssqk_ps = pps.tile([1, 512], F32, name="ssqk", tag="ssqk")
                            for co in range(4):
                                ps = pp.tile([128, 512], F32, name="psk", tag="ps")
                                for kc in range(16):
                                    nc.tensor.matmul(
                                        ps[:],
                                        wkt[:, kc, co * 128:(co + 1) * 128],
                                        xs[kc][:],
                                        start=(kc == 0),
                                        stop=(kc == 15),
                                    )
                                ksb = pe.tile([128, 512], F32, name="ksb", tag="ksb")
                                nc.scalar.activation(
                                    ksb[:], ps[:], AF.Copy, scale=knw_t[:, co:co + 1]
                                )
                                nc.sync.dma_start(
                                    r3(kTs)[co, :, tk * 512:(tk + 1) * 512], ksb[:]
                                )
                                sq = pt.tile([128, 512], F32R, name="sqk", tag="sq")
                                nc.scalar.activation(sq[:], ps[:], AF.Square)
                                nc.tensor.matmul(
                                    ssqk_ps[:],
                                    onesk_t[:],
                                    sq[:],
                                    start=(co == 0),
                                    stop=(co == 3),
                                )
                            sd = pe.tile([1, 512], F32, name="sdk", tag="sdk")
                            nc.scalar.activation(
                                sd[:], ssqk_ps[:], AF.Sqrt, bias=eps_t[:]
                            )
                            nc.vector.reciprocal(rs_k[:, tk * 512:(tk + 1) * 512], sd[:])
                            for vt in range(4):
                                ps = pp.tile([128, 512], F32, name="psv", tag="ps")
                                for kc in range(16):
                                    nc.tensor.matmul(
                                        ps[:],
                                        xs[kc][:, vt * 128:(vt + 1) * 128],
                                        wvt[:, kc, :],
                                        start=(kc == 0),
                                        stop=(kc == 15),
                                    )
                                vsb = pe.tile([128, 512], F32R, name="vsb", tag="vsb")
                                nc.scalar.activation(vsb[:], ps[:], AF.Copy)
                                nc.sync.dma_start(r3(vs)[tk * 4 + vt, :, :], vsb[:])

                    pwk.release()
                    with tc.tile_pool(name="tabraw", bufs=1) as praw:
                        bcq = praw.tile([128, TQ], F32, name="bcq")
                        nc.gpsimd.partition_broadcast(bcq[:], rs_q[:])
                        bck = praw.tile([128, T], F32, name="bck")
                        nc.gpsimd.partition_broadcast(bck[:], rs_k[:])
                        nc.vector.tensor_mul(c2qs[:], c2qs[:], bcq[:])
                        nc.vector.tensor_mul(s2qs[:], s2qs[:], bcq[:])
                        nc.vector.tensor_mul(c2ks[:], c2ks[:], bck[:])
                        nc.vector.tensor_mul(s2ks[:], s2ks[:], bck[:])

                    # ---------------- P2: attention ----------------
                    with tc.tile_pool(name="kg", bufs=1) as pkg, \
                         tc.tile_pool(name="krp", bufs=2) as pkr, \
                         tc.tile_pool(name="vg", bufs=2) as pvg, \
                         tc.tile_pool(name="qh", bufs=2) as pqh, \
                         tc.tile_pool(name="Sp", bufs=2) as pS, \
                         tc.tile_pool(name="yev", bufs=3) as pye, \
                         tc.tile_pool(name="sps", bufs=2, space="PSUM") as ppS, \
                         tc.tile_pool(name="denp", bufs=2, space="PSUM") as ppd, \
                         tc.tile_pool(name="ytp", bufs=2, space="PSUM") as ppy:
                        for g in range(N_KV_HEAD):
                            kA = pkg.tile([128, T], F32, name="kA", tag="kA")
                            nc.sync.dma_start(kA[:], r3(kTs)[g])
                            kS = pkg.tile([128, T], F32, name="kS", tag="kS")
                            nc.sync.dma_start(kS[0:64, :], r3(kTs)[g, 64:128, :])
                            nc.sync.dma_start(kS[64:128, :], r3(kTs)[g, 0:64, :])
                            nc.vector.tensor_mul(kA[:], kA[:], c2ks[:])
                            nc.vector.tensor_mul(kS[:], kS[:], s2ks[:])
                            kR = pkr.tile([128, T], F32R, name="kR", tag="kR")
                            nc.vector.tensor_add(kR[:], kA[:], kS[:])
                            vR = pvg.tile([128, 16, 128], F32R, name="vR", tag="vR")
                            nc.sync.dma_start(
                                vR[:], rp(vs)[:, :, g * 128:(g + 1) * 128]
                            )
                            for h in range(g * 4, g * 4 + 4):
                                qA = pqh.tile([128, TQ], F32, name="qA", tag="qA")
                                nc.sync.dma_start(qA[:], r3(qTs)[h])
                                qS = pqh.tile([128, TQ], F32, name="qS", tag="qS")
                                nc.sync.dma_start(qS[0:64, :], r3(qTs)[h, 64:128, :])
                                nc.sync.dma_start(qS[64:128, :], r3(qTs)[h, 0:64, :])
                                nc.vector.tensor_mul(qA[:], qA[:], c2qs[:])
                                nc.vector.tensor_mul(qS[:], qS[:], s2qs[:])
                                qR = pqh.tile([128, TQ], F32R, name="qR", tag="qR")
                                nc.vector.tensor_add(qR[:], qA[:], qS[:])
                                for qc in range(2):
                                    S_sb = pS.tile(
                                        [128, 16, 512], F32R, name="S_sb", tag="S"
                                    )
                                    for j in range(8):
                                        sps = ppS.tile(
                                            [128, 2, 512], F32, name="sps", tag="sps"
                                        )
                                        for i in range(2):
                                            kc = 2 * j + i
                                            nc.tensor.matmul(
                                                sps[:, i, :],
                                                kR[:, kc * 128:(kc + 1) * 128],
                                                qR[:, qc * 512:(qc + 1) * 512],
                                                start=True,
                                                stop=True,
                                            )
                                        nc.scalar.activation(
                                            S_sb[:, 2 * j:2 * j + 2, :], sps[:], AF.Exp
                                        )
                                    den_ps = ppd.tile([1, 512], F32, name="den", tag="den")
                                    yt_ps = ppy.tile([128, 512], F32, name="ytp", tag="ytp")
                                    for kc in range(16):
                                        nc.tensor.matmul(
                                            den_ps[:],
                                            ones_t[:],
                                            S_sb[:, kc, :],
                                            start=(kc == 0),
                                            stop=(kc == 15),
                                        )
                                        nc.tensor.matmul(
                                            yt_ps[:],
                                            vR[:, kc, :],
                                            S_sb[:, kc, :],
                                            start=(kc == 0),
                                            stop=(kc == 15),
                                        )
                                    rcp = pye.tile([1, 512], F32, name="rcp", tag="rcp")
                                    nc.vector.reciprocal(rcp[:], den_ps[:])
                                    bcr = pye.tile([128, 512], F32, name="bcr", tag="bcr")
                                    nc.gpsimd.partition_broadcast(bcr[:], rcp[:])
                                    yT_sb = pye.tile(
                                        [128, 512], F32R, name="yT_sb", tag="yT_sb"
                                    )
                                    nc.vector.tensor_mul(yT_sb[:], yt_ps[:], bcr[:])
                                    nc.sync.dma_start(
                                        r3(yTs)[h, :, qc * 512:(qc + 1) * 512], yT_sb[:]
                                    )

                # ---------------- P3: output projection ----------------
                with tc.tile_pool(name="yTf", bufs=1) as pyt, \
                     tc.tile_pool(name="woc", bufs=2) as pwo, \
                     tc.tile_pool(name="ev3", bufs=4) as pe3, \
                     tc.tile_pool(name="pp3", bufs=4, space="PSUM") as pp3:
                    yTf = pyt.tile([128, 16, TQ], F32R, name="yTf")
                    for yc in range(16):
                        nc.sync.dma_start(yTf[:, yc, :], rp(yTs)[:, yc, :])
                    for co in range(4):
                        woc = pwo.tile([128, 16, 512], F32R, name="woc", tag="woc")
                        for yc in range(16):
                            nc.sync.dma_start(
                                woc[:, yc, :],
                                rp(wo)[:, yc, co * 512:(co + 1) * 512].bitcast(F32R),
                            )
                        for qt in range(8):
                            ps = pp3.tile([128, 512], F32, name="pso", tag="ps")
                            for yc in range(16):
                                nc.tensor.matmul(
                                    ps[:],
                                    yTf[:, yc, qt * 128:(qt + 1) * 128],
                                    woc[:, yc, :],
                                    start=(yc == 0),
                                    stop=(yc == 15),
                                )
                            osb = pe3.tile([128, 512], F32, name="osb", tag="osb")
                            nc.scalar.activation(osb[:], ps[:], AF.Copy)
                            nc.sync.dma_start(
                                out[qt * 128:(qt + 1) * 128, co * 512:(co + 1) * 512],
                                osb[:],
                            )

    nc.compile()
    return nc


def _make_in_maps(inputs):
    x = np.asarray(inputs["x"], np.float32)
    cos = np.asarray(inputs["cos"], np.float32)
    sin = np.asarray(inputs["sin"], np.float32)
    wq = np.ascontiguousarray(np.asarray(inputs["wq"], np.float32))
    wk = np.ascontiguousarray(np.asarray(inputs["wk"], np.float32))
    wv = np.ascontiguousarray(np.asarray(inputs["wv"], np.float32))
    wo = np.ascontiguousarray(np.asarray(inputs["wo"], np.float32))
    qnw = np.ascontiguousarray(
        np.asarray(inputs["q_norm_w"], np.float32).reshape(16, 128).T
    )
    knw = np.ascontiguousarray(
        np.asarray(inputs["k_norm_w"], np.float32).reshape(4, 128).T
    )

    cf = cos[0, :, 0, :].T  # (64, T)
    sf = sin[0, :, 0, :].T
    c2k = np.ascontiguousarray(np.concatenate([cf, cf], 0))  # (128, T)
    s2k = np.ascontiguousarray(np.concatenate([sf, -sf], 0))
    scale = 1.0 / np.sqrt(np.float32(HEAD_DIM))

    in_maps = []
    for c in range(N_CORES):
        b, r0 = c // 2, (c % 2) * TQ
        xT = np.ascontiguousarray(x[b].T)
        in_maps.append({
            "xT": xT,
            "xTq": np.ascontiguousarray(xT[:, r0:r0 + TQ]),
            "wq": wq, "wk": wk, "wv": wv, "wo": wo,
            "c2q": np.ascontiguousarray(c2k[:, r0:r0 + TQ] * scale),
            "s2q": np.ascontiguousarray(s2k[:, r0:r0 + TQ] * scale),
            "c2k": c2k, "s2k": s2k,
            "qnw": qnw, "knw": knw,
        })
    return in_maps


def run(inputs, **spmd_kwargs):
    from concourse import bass_utils

    if "nc" not in _CACHE:
        _CACHE["nc"] = _build_nc()
    nc = _CACHE["nc"]
    res = bass_utils.run_bass_kernel_spmd(
        nc, _make_in_maps(inputs), core_ids=list(range(N_CORES)), **spmd_kwargs
    )
    out = np.empty((B, T, C), np.float32)
    for c in range(N_CORES):
        b, r0 = c // 2, (c % 2) * TQ
        out[b, r0:r0 + TQ, :] = res.results[c]["out"]
    return out, res


def kernel(**inputs):
    out, _ = run(inputs)
    return out



# revision 12
# speedup vs baseline: 1.0533x; 1.0533x over previous
"""Bidirectional GQA attention block (B=4,T=2048,C=2048,H=16,KVH=4) on 8 TRN2 cores.

Sharding: data-parallel over (batch, seq-half): core c handles batch b=c//2,
query tokens [r0, r0+1024) with r0=(c%2)*1024.  k/v are computed for the full
batch sequence on each core (2x duplicated k/v-proj work, ~8% overhead) so no
cross-core communication is needed; the final output is a pure concatenation.

v2 pipeline (everything staged in fp16; all matmuls fp16 at full PE rate;
PSUM accumulation fp32):
  P1a: q^T = wq^T x^T channel-major -> qTs (DRAM, fp16).  Sum-of-squares for
       RMSNorm via (1/C)-ones matmul; rs = 1/sqrt(mean+eps) (ACT sqrt + DVE
       recip), folded into per-token rope tables (q tables pre-scaled by
       1/sqrt(head_dim) on host).
  P1b: k^T and v projections written DIRECTLY into SBUF-resident tiles
       (no DRAM round trip).  Rope k-tables scaled per 512-token chunk.
  P2:  per kv-group g: kR = kT*c2k + kTswap*s2k (swap via SBUF->SBUF DMA);
       per head: qR likewise (q re-read from DRAM).  logits^T = kR_chunk qR
       per 128-key chunk, exp on ACT -> S fp16.  Softmax denominator via a
       4-level DVE pairwise tree (fp16, 2x mode) + ONE 512-row ones-matmul
       per block (16x less PE than the naive ones-matmul).  den rows for the
       8 blocks of a group batch into one [8,512] PSUM tile -> ONE DVE
       reciprocal per group.  y^T = v S accumulated in PSUM, staged to SBUF
       (ACT), divided by den (DVE) into the SBUF-resident yT tile.
  P3:  out = yT.T wo with PSUM accumulation over the 16 head-chunks.
"""
import sys
import os

sys.path.insert(0, "/opt/trn_rl_repo")

import numpy as np

B, T, C = 4, 2048, 2048
N_HEAD, N_KV_HEAD = 16, 4
HEAD_DIM = C // N_HEAD  # 128
KV_DIM = N_KV_HEAD * HEAD_DIM  # 512
EPS = 1e-5
TQ = 1024  # query tokens per core
N_CORES = 8

_CACHE = {}


def _build_nc(reps=1, trace_sim=False):
    import concourse.bass as bass
    import concourse.mybir as mybir
    import concourse.tile as tile
    from concourse import bacc

    F32 = mybir.dt.float32
    F16 = mybir.dt.float16
    AF = mybir.ActivationFunctionType

    nc = bacc.Bacc("TRN2", target_bir_lowering=False, debug=False)

    def ein(name, shape, dt=F16):
        return nc.dram_tensor(name, shape, dt, kind="ExternalInput").ap()

    xT = ein("xT", [C, T])          # x[b].T  (c_in, tok) fp16
    xTq = ein("xTq", [C, TQ])       # x[b].T[:, r0:r0+TQ] fp16
    wq = ein("wq", [C, C])
    wk = ein("wk", [C, KV_DIM])
    wv = ein("wv", [C, KV_DIM])
    wo = ein("wo", [C, C])
    c2q = ein("c2q", [128, TQ])     # [cos;cos] / sqrt(HEAD_DIM), q token slice
    s2q = ein("s2q", [128, TQ])     # [sin;-sin] / sqrt(HEAD_DIM)
    c2k = ein("c2k", [128, T])      # [cos;cos] (unscaled)
    s2k = ein("s2k", [128, T])
    qnw = ein("qnw", [128, 16], F32)  # q_norm_w.reshape(16,128).T
    knw = ein("knw", [128, 4], F32)
    out = nc.dram_tensor("out", [TQ, C], F32, kind="ExternalOutput").ap()

    ones_d = nc.inline_tensor(np.ones((128, 1), np.float16), name="onesc").ap()
    onesq_d = nc.inline_tensor(
        np.full((128, 1), 1.0 / C, np.float16), name="onesqc"
    ).ap()
    onesk_d = nc.inline_tensor(
        np.full((128, 1), 1.0 / KV_DIM, np.float16), name="oneskc"
    ).ap()
    eps_d = nc.inline_tensor(np.full((1, 1), EPS, np.float32), name="epsc").ap()

    # DRAM scratch: only q^T is staged (k/v/y live in SBUF)
    qTs = nc.dram_tensor("qTs", [C, TQ], F16).ap()

    def r3(ap, p=128):
        # (c*p, n) -> (c, p, n)
        return ap.rearrange("(c p) n -> c p n", p=p)

    def rp(ap, p=128):
        # (c*p, n) -> (p, c, n)
        return ap.rearrange("(c p) n -> p c n", p=p)

    # state shared between phases of one rep
    st = {}

    def p1a(tc, cs):
        """q^T projection + rmsnorm stats; loads rope tables; allocates
        SBUF-resident k/v/y tiles."""
        with tc.tile_pool(name="p1q", bufs=1) as p1, \
             tc.tile_pool(name="wqlp", bufs=2) as pw, \
             tc.tile_pool(name="ev1", bufs=3) as pe, \
             tc.tile_pool(name="tmp1", bufs=3) as pt, \
             tc.tile_pool(name="rsp", bufs=2) as prs, \
             tc.tile_pool(name="pp1", bufs=4, space="PSUM") as pp, \
             tc.tile_pool(name="ssqp", bufs=1, space="PSUM") as pps:
            xqs = []
            for tq in range(2):
                xq = p1.tile([128, 16, 512], F16, name=f"xq{tq}", tag=f"xq{tq}")
                nc.sync.dma_start(xq[:], rp(xTq)[:, :, tq * 512:(tq + 1) * 512])
                xqs.append(xq)
            # rope tables (scaled at end of P1a / in P1b)
            for nm, src in (("c2qs", c2q), ("s2qs", s2q),
                            ("c2ks", c2k), ("s2ks", s2k)):
                t = st["ptab"].tile([128, src.shape[-1]], F16, name=nm)
                nc.sync.dma_start(t[:], src)
                st[nm] = t
            # SBUF-resident k/v/y
            st["kT"] = st["pres"].tile([128, 4, T], F16, name="kT_res")
            st["v"] = st["pres"].tile([128, 16, KV_DIM], F16, name="v_res")
            st["yT"] = st["pres"].tile([128, 16, TQ], F16, name="yT_res")

            ssq_ps = [
                pps.tile([1, 512], F32, name=f"ssqq{tq}", tag=f"ssqq{tq}")
                for tq in range(2)
            ]
            for cout in range(16):
                wql = pw.tile([128, 16, 128], F16, name="wql", tag="wql")
                nc.sync.dma_start(
                    wql[:], rp(wq)[:, :, cout * 128:(cout + 1) * 128]
                )
                for tq in range(2):
                    ps = pp.tile([128, 512], F32, name="psq", tag="ps")
                    for kc in range(16):
                        nc.tensor.matmul(
                            ps[:], wql[:, kc, :], xqs[tq][:, kc, :],
                            start=(kc == 0), stop=(kc == 15),
                        )
                    qsb = pe.tile([128, 512], F16, name="qsb", tag="qsb")
                    nc.scalar.activation(
                        qsb[:], ps[:], AF.Copy, scale=cs["qnw"][:, cout:cout + 1]
                    )
                    nc.sync.dma_start(
                        r3(qTs)[cout, :, tq * 512:(tq + 1) * 512], qsb[:]
                    )
                    sq = pt.tile([128, 512], F16, name="sqq", tag="sq")
                    nc.scalar.activation(sq[:], ps[:], AF.Square)
                    nc.tensor.matmul(
                        ssq_ps[tq][:], cs["onesq"][:], sq[:],
                        start=(cout == 0), stop=(cout == 15),
                    )
            for tq in range(2):
                sl = slice(tq * 512, (tq + 1) * 512)
                sd = prs.tile([1, 512], F32, name="sdq", tag="sdq")
                nc.scalar.activation(sd[:], ssq_ps[tq][:], AF.Sqrt,
                                     bias=cs["eps"][:])
                rs = prs.tile([1, 512], F32, name="rsq", tag="rsq")
                nc.vector.reciprocal(rs[:], sd[:])
                bcq = prs.tile([128, 512], F32, name="bcq", tag="bcq")
                nc.gpsimd.partition_broadcast(bcq[:], rs[:])
                nc.vector.tensor_mul(st["c2qs"][:, sl], st["c2qs"][:, sl], bcq[:])
                nc.vector.tensor_mul(st["s2qs"][:, sl], st["s2qs"][:, sl], bcq[:])

    def p1b(tc, cs):
        """k^T and v projections into SBUF-resident tiles + k-table scaling."""
        kT_res, v_res = st["kT"], st["v"]
        with tc.tile_pool(name="wkv", bufs=1) as pwkv, \
             tc.tile_pool(name="xsp", bufs=2) as pxs, \
             tc.tile_pool(name="tmp2", bufs=3) as pt, \
             tc.tile_pool(name="rsk", bufs=2) as prs, \
             tc.tile_pool(name="pp2", bufs=4, space="PSUM") as pp, \
             tc.tile_pool(name="ssqk", bufs=2, space="PSUM") as pps:
            wkt = pwkv.tile([128, 16, KV_DIM], F16, name="wkt")
            nc.sync.dma_start(wkt[:], rp(wk))
            wvt = pwkv.tile([128, 16, KV_DIM], F16, name="wvt")
            nc.sync.dma_start(wvt[:], rp(wv))
            for tk in range(4):
                tsl = slice(tk * 512, (tk + 1) * 512)
                xs = pxs.tile([128, 16, 512], F16, name="xsc", tag="xsc")
                nc.sync.dma_start(xs[:], rp(xT)[:, :, tsl])
                ssqk_ps = pps.tile([1, 512], F32, name="ssqk", tag="ssqk")
                for co in range(4):
                    ps = pp.tile([128, 512], F32, name="psk", tag="ps")
                    for kc in range(16):
                        nc.tensor.matmul(
                            ps[:], wkt[:, kc, co * 128:(co + 1) * 128], xs[:, kc, :],
                            start=(kc == 0), stop=(kc == 15),
                        )
                    nc.scalar.activation(
                        kT_res[:, co, tsl], ps[:], AF.Copy,
                        scale=cs["knw"][:, co:co + 1]
                    )
                    sq = pt.tile([128, 512], F16, name="sqk", tag="sq")
                    nc.scalar.activation(sq[:], ps[:], AF.Square)
                    nc.tensor.matmul(
                        ssqk_ps[:], cs["onesk"][:], sq[:],
                        start=(co == 0), stop=(co == 3),
                    )
                sd = prs.tile([1, 512], F32, name="sdk", tag="sdk")
                nc.scalar.activation(sd[:], ssqk_ps[:], AF.Sqrt, bias=cs["eps"][:])
                rs = prs.tile([1, 512], F32, name="rsk", tag="rsk")
                nc.vector.reciprocal(rs[:], sd[:])
                bck = prs.tile([128, 512], F32, name="bck", tag="bck")
                nc.gpsimd.partition_broadcast(bck[:], rs[:])
                nc.vector.tensor_mul(st["c2ks"][:, tsl], st["c2ks"][:, tsl], bck[:])
                nc.vector.tensor_mul(st["s2ks"][:, tsl], st["s2ks"][:, tsl], bck[:])
                for vt in range(4):
                    ps = pp.tile([128, 512], F32, name="psv", tag="ps")
                    for kc in range(16):
                        nc.tensor.matmul(
                            ps[:], xs[:, kc, vt * 128:(vt + 1) * 128], wvt[:, kc, :],
                            start=(kc == 0), stop=(kc == 15),
                        )
                    nc.scalar.activation(v_res[:, tk * 4 + vt, :], ps[:], AF.Copy)

    def p2_block(tc, cs, pools, g, hh, qc, qR, kR):
        """One (head, qc) attention block: S matmuls + exp + den tree + AV +
        normalization (1/den via exp(-ln(den)) on ACT: ln and exp co-reside
        in the natural_log_exp_and_others table, so no table thrash and no
        slow DVE reciprocal)."""
        pS, p8, p4, p2t, p1t, prc, pbc, ppS, ppd, ppy = pools
        h = g * 4 + hh
        qsl = slice(qc * 512, (qc + 1) * 512)
        S_sb = pS.tile([128, 16, 512], F16, name="S_sb", tag="S")
        for j in range(8):
            sps = ppS.tile([128, 2, 512], F32, name="sps", tag="sps")
            for i in range(2):
                kc = 2 * j + i
                nc.tensor.matmul(
                    sps[:, i, :], kR[:, kc * 128:(kc + 1) * 128], qR[:, qsl],
                    start=True, stop=True,
                )
            nc.scalar.activation(S_sb[:, 2 * j:2 * j + 2, :], sps[:], AF.Exp)
        # denominator: 4-level pairwise tree on DVE (fp16, 2x mode)
        t8 = p8.tile([128, 8, 512], F16, name="t8", tag="t8")
        nc.vector.tensor_add(t8[:], S_sb[:, 0:8, :], S_sb[:, 8:16, :])
        t4 = p4.tile([128, 4, 512], F16, name="t4", tag="t4")
        nc.vector.tensor_add(t4[:], t8[:, 0:4, :], t8[:, 4:8, :])
        t2 = p2t.tile([128, 2, 512], F16, name="t2", tag="t2")
        nc.vector.tensor_add(t2[:], t4[:, 0:2, :], t4[:, 2:4, :])
        t1 = p1t.tile([128, 512], F16, name="t1", tag="t1")
        nc.vector.tensor_add(t1[:], t2[:, 0, :], t2[:, 1, :])
        den_ps = ppd.tile([1, 512], F32, name="den", tag="den")
        nc.tensor.matmul(den_ps[:], cs["ones"][:], t1[:], start=True, stop=True)
        lnd = prc.tile([1, 512], F32, name="lnd", tag="lnd")
        nc.scalar.activation(lnd[:], den_ps[:], AF.Ln)
        rcp = prc.tile([1, 512], F32, name="rcp", tag="rcp")
        nc.scalar.activation(rcp[:], lnd[:], AF.Exp, scale=-1.0)
        bcr = pbc.tile([128, 512], F32, name="bcr", tag="bcr")
        nc.gpsimd.partition_broadcast(bcr[:], rcp[:])
        yt_ps = ppy.tile([128, 512], F32, name="ytp", tag="ytp")
        g128 = slice(g * 128, (g + 1) * 128)
        for kc in range(16):
            nc.tensor.matmul(
                yt_ps[:], st["v"][:, kc, g128], S_sb[:, kc, :],
                start=(kc == 0), stop=(kc == 15),
            )
        nc.vector.tensor_mul(
            st["yT"][:, h, qsl], yt_ps[:], bcr[:],
        )

    def p2(tc, cs):
        """attention over 4 kv-groups x 4 heads x 2 q-chunks."""
        kT_res, yT_res = st["kT"], st["yT"]
        c2qs, s2qs, c2ks, s2ks = st["c2qs"], st["s2qs"], st["c2ks"], st["s2ks"]
        with tc.tile_pool(name="ksw", bufs=2) as pks, \
             tc.tile_pool(name="krp", bufs=2) as pkr, \
             tc.tile_pool(name="qh", bufs=2) as pqh, \
             tc.tile_pool(name="Sp", bufs=2) as pS, \
             tc.tile_pool(name="tr8", bufs=2) as p8, \
             tc.tile_pool(name="tr4", bufs=2) as p4, \
             tc.tile_pool(name="tr2", bufs=2) as p2t, \
             tc.tile_pool(name="tr1", bufs=2) as p1t, \
             tc.tile_pool(name="rcb", bufs=3) as prc, \
             tc.tile_pool(name="bcb", bufs=2) as pbc, \
             tc.tile_pool(name="sps", bufs=2, space="PSUM") as ppS, \
             tc.tile_pool(name="dnp", bufs=2, space="PSUM") as ppd, \
             tc.tile_pool(name="ytp", bufs=2, space="PSUM") as ppy:
            pools = (pS, p8, p4, p2t, p1t, prc, pbc, ppS, ppd, ppy)
            for g in range(N_KV_HEAD):
                kSw = pks.tile([128, T], F16, name="kSw", tag="kSw")
                nc.sync.dma_start(kSw[0:64, :], kT_res[64:128, g, :])
                nc.sync.dma_start(kSw[64:128, :], kT_res[0:64, g, :])
                kA = pkr.tile([128, T], F16, name="kA", tag="kA")
                nc.vector.tensor_mul(kA[:], kT_res[:, g, :], c2ks[:])
                nc.vector.tensor_mul(kSw[:], kSw[:], s2ks[:])
                kR = pkr.tile([128, T], F16, name="kR", tag="kR")
                nc.vector.tensor_add(kR[:], kA[:], kSw[:])
                for hh in range(4):
                    h = g * 4 + hh
                    qTt = pqh.tile([128, TQ], F16, name="qTt", tag="qTt")
                    nc.sync.dma_start(qTt[:], r3(qTs)[h])
                    qSw = pqh.tile([128, TQ], F16, name="qSw", tag="qSw")
                    nc.sync.dma_start(qSw[0:64, :], r3(qTs)[h, 64:128, :])
                    nc.sync.dma_start(qSw[64:128, :], r3(qTs)[h, 0:64, :])
                    qA = pqh.tile([128, TQ], F16, name="qA", tag="qA")
                    nc.vector.tensor_mul(qA[:], qTt[:], c2qs[:])
                    nc.vector.tensor_mul(qSw[:], qSw[:], s2qs[:])
                    qR = pqh.tile([128, TQ], F16, name="qR", tag="qR")
                    nc.vector.tensor_add(qR[:], qA[:], qSw[:])
                    for qc in range(2):
                        p2_block(tc, cs, pools, g, hh, qc, qR, kR)

    def p3(tc, cs):
        """output projection out = yT.T @ wo."""
        yT_res = st["yT"]
        with tc.tile_pool(name="woc", bufs=2) as pwo, \
             tc.tile_pool(name="ev3", bufs=4) as pe3, \
             tc.tile_pool(name="pp3", bufs=4, space="PSUM") as pp3:
            for co in range(4):
                woc = pwo.tile([128, 16, 512], F16, name="woc", tag="woc")
                nc.sync.dma_start(woc[:], rp(wo)[:, :, co * 512:(co + 1) * 512])
                for qt in range(8):
                    ps = pp3.tile([128, 512], F32, name="pso", tag="ps")
                    for yc in range(16):
                        nc.tensor.matmul(
                            ps[:], yT_res[:, yc, qt * 128:(qt + 1) * 128],
                            woc[:, yc, :],
                            start=(yc == 0), stop=(yc == 15),
                        )
                    osb = pe3.tile([128, 512], F32, name="osb", tag="osb")
                    nc.scalar.activation(osb[:], ps[:], AF.Copy)
                    nc.sync.dma_start(
                        out[qt * 128:(qt + 1) * 128, co * 512:(co + 1) * 512],
                        osb[:],
                    )

    with tile.TileContext(nc, trace_sim=trace_sim) as tc:
        with tc.tile_pool(name="const", bufs=1) as cpool:
            cs = {}
            for nm, src in (("ones", ones_d), ("onesq", onesq_d),
                            ("onesk", onesk_d)):
                t = cpool.tile([128, 1], F16, name=nm + "_t")
                nc.sync.dma_start(t[:], src)
                cs[nm] = t
            cs["eps"] = cpool.tile([1, 1], F32, name="eps_t")
            nc.sync.dma_start(cs["eps"][:], eps_d)
            cs["qnw"] = cpool.tile([128, 16], F32, name="qnw_t")
            nc.sync.dma_start(cs["qnw"][:], qnw)
            cs["knw"] = cpool.tile([128, 4], F32, name="knw_t")
            nc.sync.dma_start(cs["knw"][:], knw)

            for rep in range(reps):
                with tc.tile_pool(name="resid", bufs=1) as pres, \
                     tc.tile_pool(name="tabs", bufs=1) as ptab:
                    st.clear()
                    st["pres"], st["ptab"] = pres, ptab
                    p1a(tc, cs)
                    p1b(tc, cs)
                    p2(tc, cs)
                    p3(tc, cs)

    nc.compile()
    return nc


def _make_in_maps(inputs):
    F16 = np.float16
    x = np.asarray(inputs["x"], np.float32)
    cos = np.asarray(inputs["cos"], np.float32)
    sin = np.asarray(inputs["sin"], np.float32)
    wq = np.ascontiguousarray(np.asarray(inputs["wq"], np.float32).astype(F16))
    wk = np.ascontiguousarray(np.asarray(inputs["wk"], np.float32).astype(F16))
    wv = np.ascontiguousarray(np.asarray(inputs["wv"], np.float32).astype(F16))
    wo = np.ascontiguousarray(np.asarray(inputs["wo"], np.float32).astype(F16))
    qnw = np.ascontiguousarray(
        np.asarray(inputs["q_norm_w"], np.float32).reshape(16, 128).T
    )
    knw = np.ascontiguousarray(
        np.asarray(inputs["k_norm_w"], np.float32).reshape(4, 128).T
    )

    cf = cos[0, :, 0, :].T  # (64, T)
    sf = sin[0, :, 0, :].T
    c2k = np.concatenate([cf, cf], 0)  # (128, T)
    s2k = np.concatenate([sf, -sf], 0)
    scale = 1.0 / np.sqrt(np.float32(HEAD_DIM))
    c2k16 = np.ascontiguousarray(c2k.astype(F16))
    s2k16 = np.ascontiguousarray(s2k.astype(F16))

    in_maps = []
    for c in range(N_CORES):
        b, r0 = c // 2, (c % 2) * TQ
        xT = np.ascontiguousarray(x[b].T.astype(F16))
        in_maps.append({
            "xT": xT,
            "xTq": np.ascontiguousarray(xT[:, r0:r0 + TQ]),
            "wq": wq, "wk": wk, "wv": wv, "wo": wo,
            "c2q": np.ascontiguousarray((c2k[:, r0:r0 + TQ] * scale).astype(F16)),
            "s2q": np.ascontiguousarray((s2k[:, r0:r0 + TQ] * scale).astype(F16)),
            "c2k": c2k16, "s2k": s2k16,
            "qnw": qnw, "knw": knw,
        })
    return in_maps


def run(inputs, **spmd_kwargs):
    from concourse import bass_utils

    if "nc" not in _CACHE:
        _CACHE["nc"] = _build_nc()
    nc = _CACHE["nc"]
    res = bass_utils.run_bass_kernel_spmd(
        nc, _make_in_maps(inputs), core_ids=list(range(N_CORES)), **spmd_kwargs
    )
    out = np.empty((B, T, C), np.float32)
    for c in range(N_CORES):
        b, r0 = c // 2, (c % 2) * TQ
        out[b, r0:r0 + TQ, :] = res.results[c]["out"]
    return out, res


def kernel(**inputs):
    out, _ = run(inputs)
    return out


# revision 13
# speedup vs baseline: 1.1482x; 1.0901x over previous
"""Bidirectional GQA attention block (B=4,T=2048,C=2048,H=16,KVH=4) on 8 TRN2 cores.

Sharding: data-parallel over (batch, seq-half): core c handles batch b=c//2,
query tokens [r0, r0+1024) with r0=(c%2)*1024.  k/v are computed for the full
batch sequence on each core (2x duplicated k/v-proj work, ~8% overhead) so no
cross-core communication is needed; the final output is a pure concatenation.

v2 pipeline (everything staged in fp16; all matmuls fp16 at full PE rate;
PSUM accumulation fp32):
  P1a: q^T = wq^T x^T channel-major -> qTs (DRAM, fp16).  Sum-of-squares for
       RMSNorm via (1/C)-ones matmul; rs = 1/sqrt(mean+eps) (ACT sqrt + DVE
       recip), folded into per-token rope tables (q tables pre-scaled by
       1/sqrt(head_dim) on host).
  P1b: k^T and v projections written DIRECTLY into SBUF-resident tiles
       (no DRAM round trip).  Rope k-tables scaled per 512-token chunk.
  P2:  per kv-group g: kR = kT*c2k + kTswap*s2k (swap via SBUF->SBUF DMA);
       per head: qR likewise (q re-read from DRAM).  logits^T = kR_chunk qR
       per 128-key chunk, exp on ACT -> S fp16.  Softmax denominator via a
       4-level DVE pairwise tree (fp16, 2x mode) + ONE 512-row ones-matmul
       per block (16x less PE than the naive ones-matmul).  den rows for the
       8 blocks of a group batch into one [8,512] PSUM tile -> ONE DVE
       reciprocal per group.  y^T = v S accumulated in PSUM, staged to SBUF
       (ACT), divided by den (DVE) into the SBUF-resident yT tile.
  P3:  out = yT.T wo with PSUM accumulation over the 16 head-chunks.
"""
import sys
import os

sys.path.insert(0, "/opt/trn_rl_repo")

import numpy as np

B, T, C = 4, 2048, 2048
N_HEAD, N_KV_HEAD = 16, 4
HEAD_DIM = C // N_HEAD  # 128
KV_DIM = N_KV_HEAD * HEAD_DIM  # 512
EPS = 1e-5
TQ = 1024  # query tokens per core
N_CORES = 8

_CACHE = {}


def _build_nc(reps=1, trace_sim=False):
    import concourse.bass as bass
    import concourse.mybir as mybir
    import concourse.tile as tile
    from concourse import bacc

    F32 = mybir.dt.float32
    F16 = mybir.dt.float16
    AF = mybir.ActivationFunctionType

    nc = bacc.Bacc("TRN2", target_bir_lowering=False, debug=False)

    def ein(name, shape, dt=F16):
        return nc.dram_tensor(name, shape, dt, kind="ExternalInput").ap()

    xT = ein("xT", [C, T])          # x[b].T  (c_in, tok) fp16
    xTq = ein("xTq", [C, TQ])       # x[b].T[:, r0:r0+TQ] fp16
    wq = ein("wq", [C, C])
    wk = ein("wk", [C, KV_DIM])
    wv = ein("wv", [C, KV_DIM])
    wo = ein("wo", [C, C])
    c2q = ein("c2q", [128, TQ])     # [cos;cos] / sqrt(HEAD_DIM), q token slice
    s2q = ein("s2q", [128, TQ])     # [sin;-sin] / sqrt(HEAD_DIM)
    c2k = ein("c2k", [128, T])      # [cos;cos] (unscaled)
    s2k = ein("s2k", [128, T])
    qnw = ein("qnw", [128, 16], F32)  # q_norm_w.reshape(16,128).T
    knw = ein("knw", [128, 4], F32)
    out = nc.dram_tensor("out", [TQ, C], F32, kind="ExternalOutput").ap()

    ones_d = nc.inline_tensor(np.ones((128, 1), np.float16), name="onesc").ap()
    onesq_d = nc.inline_tensor(
        np.full((128, 1), 1.0 / C, np.float16), name="onesqc"
    ).ap()
    onesk_d = nc.inline_tensor(
        np.full((128, 1), 1.0 / KV_DIM, np.float16), name="oneskc"
    ).ap()
    eps_d = nc.inline_tensor(np.full((1, 1), EPS, np.float32), name="epsc").ap()

    # DRAM scratch: only q^T is staged (k/v/y live in SBUF)
    qTs = nc.dram_tensor("qTs", [C, TQ], F16).ap()

    def r3(ap, p=128):
        # (c*p, n) -> (c, p, n)
        return ap.rearrange("(c p) n -> c p n", p=p)

    def rp(ap, p=128):
        # (c*p, n) -> (p, c, n)
        return ap.rearrange("(c p) n -> p c n", p=p)

    # state shared between phases of one rep
    st = {}

    def p1a(tc, cs):
        """q^T projection + rmsnorm stats; loads rope tables; allocates
        SBUF-resident k/v/y tiles."""
        with tc.tile_pool(name="p1q", bufs=1) as p1, \
             tc.tile_pool(name="wqlp", bufs=2) as pw, \
             tc.tile_pool(name="ev1", bufs=3) as pe, \
             tc.tile_pool(name="tmp1", bufs=3) as pt, \
             tc.tile_pool(name="rsp", bufs=2) as prs, \
             tc.tile_pool(name="pp1", bufs=4, space="PSUM") as pp, \
             tc.tile_pool(name="ssqp", bufs=1, space="PSUM") as pps:
            xqs = []
            for tq in range(2):
                xq = p1.tile([128, 16, 512], F16, name=f"xq{tq}", tag=f"xq{tq}")
                nc.sync.dma_start(xq[:], rp(xTq)[:, :, tq * 512:(tq + 1) * 512])
                xqs.append(xq)
            # rope tables (scaled at end of P1a / in P1b)
            for nm, src in (("c2qs", c2q), ("s2qs", s2q),
                            ("c2ks", c2k), ("s2ks", s2k)):
                t = st["ptab"].tile([128, src.shape[-1]], F16, name=nm)
                nc.sync.dma_start(t[:], src)
                st[nm] = t
            # SBUF-resident k/v/y
            st["kT"] = st["pres"].tile([128, 4, T], F16, name="kT_res")
            st["v"] = st["pres"].tile([128, 16, KV_DIM], F16, name="v_res")
            st["yT"] = st["pres"].tile([128, 16, TQ], F16, name="yT_res")

            ssq_ps = [
                pps.tile([1, 512], F32, name=f"ssqq{tq}", tag=f"ssqq{tq}")
                for tq in range(2)
            ]
            for cout in range(16):
                wql = pw.tile([128, 16, 128], F16, name="wql", tag="wql")
                nc.sync.dma_start(
                    wql[:], rp(wq)[:, :, cout * 128:(cout + 1) * 128]
                )
                for tq in range(2):
                    ps = pp.tile([128, 512], F32, name="psq", tag="ps")
                    for kc in range(16):
                        nc.tensor.matmul(
                            ps[:], wql[:, kc, :], xqs[tq][:, kc, :],
                            start=(kc == 0), stop=(kc == 15),
                        )
                    qsb = pe.tile([128, 512], F16, name="qsb", tag="qsb")
                    nc.scalar.activation(
                        qsb[:], ps[:], AF.Copy, scale=cs["qnw"][:, cout:cout + 1]
                    )
                    nc.sync.dma_start(
                        r3(qTs)[cout, :, tq * 512:(tq + 1) * 512], qsb[:]
                    )
                    sq = pt.tile([128, 512], F16, name="sqq", tag="sq")
                    nc.scalar.activation(sq[:], ps[:], AF.Square)
                    nc.tensor.matmul(
                        ssq_ps[tq][:], cs["onesq"][:], sq[:],
                        start=(cout == 0), stop=(cout == 15),
                    )
            for tq in range(2):
                sl = slice(tq * 512, (tq + 1) * 512)
                sd = prs.tile([1, 512], F32, name="sdq", tag="sdq")
                nc.scalar.activation(sd[:], ssq_ps[tq][:], AF.Sqrt,
                                     bias=cs["eps"][:])
                rs = prs.tile([1, 512], F32, name="rsq", tag="rsq")
                nc.vector.reciprocal(rs[:], sd[:])
                bcq = prs.tile([128, 512], F32, name="bcq", tag="bcq")
                nc.gpsimd.partition_broadcast(bcq[:], rs[:])
                nc.vector.tensor_mul(st["c2qs"][:, sl], st["c2qs"][:, sl], bcq[:])
                nc.vector.tensor_mul(st["s2qs"][:, sl], st["s2qs"][:, sl], bcq[:])

    def p1b(tc, cs):
        """k^T and v projections into SBUF-resident tiles + k-table scaling."""
        kT_res, v_res = st["kT"], st["v"]
        with tc.tile_pool(name="wkv", bufs=1) as pwkv, \
             tc.tile_pool(name="xsp", bufs=2) as pxs, \
             tc.tile_pool(name="tmp2", bufs=3) as pt, \
             tc.tile_pool(name="rsk", bufs=2) as prs, \
             tc.tile_pool(name="pp2", bufs=4, space="PSUM") as pp, \
             tc.tile_pool(name="ssqk", bufs=2, space="PSUM") as pps:
            wkt = pwkv.tile([128, 16, KV_DIM], F16, name="wkt")
            nc.sync.dma_start(wkt[:], rp(wk))
            wvt = pwkv.tile([128, 16, KV_DIM], F16, name="wvt")
            nc.sync.dma_start(wvt[:], rp(wv))
            for tk in range(4):
                tsl = slice(tk * 512, (tk + 1) * 512)
                xs = pxs.tile([128, 16, 512], F16, name="xsc", tag="xsc")
                nc.sync.dma_start(xs[:], rp(xT)[:, :, tsl])
                ssqk_ps = pps.tile([1, 512], F32, name="ssqk", tag="ssqk")
                for co in range(4):
                    ps = pp.tile([128, 512], F32, name="psk", tag="ps")
                    for kc in range(16):
                        nc.tensor.matmul(
                            ps[:], wkt[:, kc, co * 128:(co + 1) * 128], xs[:, kc, :],
                            start=(kc == 0), stop=(kc == 15),
                        )
                    nc.scalar.activation(
                        kT_res[:, co, tsl], ps[:], AF.Copy,
                        scale=cs["knw"][:, co:co + 1]
                    )
                    sq = pt.tile([128, 512], F16, name="sqk", tag="sq")
                    nc.scalar.activation(sq[:], ps[:], AF.Square)
                    nc.tensor.matmul(
                        ssqk_ps[:], cs["onesk"][:], sq[:],
                        start=(co == 0), stop=(co == 3),
                    )
                sd = prs.tile([1, 512], F32, name="sdk", tag="sdk")
                nc.scalar.activation(sd[:], ssqk_ps[:], AF.Sqrt, bias=cs["eps"][:])
                rs = prs.tile([1, 512], F32, name="rsk", tag="rsk")
                nc.vector.reciprocal(rs[:], sd[:])
                bck = prs.tile([128, 512], F32, name="bck", tag="bck")
                nc.gpsimd.partition_broadcast(bck[:], rs[:])
                nc.vector.tensor_mul(st["c2ks"][:, tsl], st["c2ks"][:, tsl], bck[:])
                nc.vector.tensor_mul(st["s2ks"][:, tsl], st["s2ks"][:, tsl], bck[:])
                for vt in range(4):
                    ps = pp.tile([128, 512], F32, name="psv", tag="ps")
                    for kc in range(16):
                        nc.tensor.matmul(
                            ps[:], xs[:, kc, vt * 128:(vt + 1) * 128], wvt[:, kc, :],
                            start=(kc == 0), stop=(kc == 15),
                        )
                    nc.scalar.activation(v_res[:, tk * 4 + vt, :], ps[:], AF.Copy)

    def p2_block(tc, cs, pools, g, hh, qc, qR, kR):
        """One (head, qc) attention block: S matmuls + exp + den tree + AV +
        normalization (1/den via exp(-ln(den)) on ACT: ln and exp co-reside
        in the natural_log_exp_and_others table, so no table thrash and no
        slow DVE reciprocal)."""
        pS, p8, p4, p2t, p1t, prc, pbc, ppS, ppd, ppy = pools
        h = g * 4 + hh
        qsl = slice(qc * 512, (qc + 1) * 512)
        S_sb = pS.tile([128, 16, 512], F16, name="S_sb", tag="S")
        for j in range(8):
            sps = ppS.tile([128, 2, 512], F32, name="sps", tag="sps")
            for i in range(2):
                kc = 2 * j + i
                nc.tensor.matmul(
                    sps[:, i, :], kR[:, kc * 128:(kc + 1) * 128], qR[:, qsl],
                    start=True, stop=True,
                )
            nc.scalar.activation(S_sb[:, 2 * j:2 * j + 2, :], sps[:], AF.Exp)
        # denominator: 4-level pairwise tree on DVE (fp16, 2x mode)
        t8 = p8.tile([128, 8, 512], F16, name="t8", tag="t8")
        nc.vector.tensor_add(t8[:], S_sb[:, 0:8, :], S_sb[:, 8:16, :])
        t4 = p4.tile([128, 4, 512], F16, name="t4", tag="t4")
        nc.vector.tensor_add(t4[:], t8[:, 0:4, :], t8[:, 4:8, :])
        t2 = p2t.tile([128, 2, 512], F16, name="t2", tag="t2")
        nc.vector.tensor_add(t2[:], t4[:, 0:2, :], t4[:, 2:4, :])
        t1 = p1t.tile([128, 512], F16, name="t1", tag="t1")
        nc.vector.tensor_add(t1[:], t2[:, 0, :], t2[:, 1, :])
        # yt accumulation first: the den matmul waits on the DVE tree, and
        # the PE executes in order -- den must not block yt.
        yt_ps = ppy.tile([128, 512], F32, name="ytp", tag="ytp")
        g128 = slice(g * 128, (g + 1) * 128)
        for kc in range(16):
            nc.tensor.matmul(
                yt_ps[:], st["v"][:, kc, g128], S_sb[:, kc, :],
                start=(kc == 0), stop=(kc == 15),
            )
        den_ps = ppd.tile([1, 512], F32, name="den", tag="den")
        nc.tensor.matmul(den_ps[:], cs["ones"][:], t1[:], start=True, stop=True)
        rcp = prc.tile([1, 512], F32, name="rcp", tag="rcp")
        nc.vector.reciprocal(rcp[:], den_ps[:])
        bcr = pbc.tile([128, 512], F32, name="bcr", tag="bcr")
        nc.gpsimd.partition_broadcast(bcr[:], rcp[:])
        nc.vector.tensor_mul(
            st["yT"][:, h, qsl], yt_ps[:], bcr[:],
        )

    def p2(tc, cs):
        """attention over 4 kv-groups x 4 heads x 2 q-chunks."""
        kT_res, yT_res = st["kT"], st["yT"]
        c2qs, s2qs, c2ks, s2ks = st["c2qs"], st["s2qs"], st["c2ks"], st["s2ks"]
        with tc.tile_pool(name="ksw", bufs=2) as pks, \
             tc.tile_pool(name="krp", bufs=2) as pkr, \
             tc.tile_pool(name="qh", bufs=2) as pqh, \
             tc.tile_pool(name="Sp", bufs=2) as pS, \
             tc.tile_pool(name="tr8", bufs=2) as p8, \
             tc.tile_pool(name="tr4", bufs=2) as p4, \
             tc.tile_pool(name="tr2", bufs=2) as p2t, \
             tc.tile_pool(name="tr1", bufs=2) as p1t, \
             tc.tile_pool(name="rcb", bufs=3) as prc, \
             tc.tile_pool(name="bcb", bufs=2) as pbc, \
             tc.tile_pool(name="sps", bufs=2, space="PSUM") as ppS, \
             tc.tile_pool(name="dnp", bufs=2, space="PSUM") as ppd, \
             tc.tile_pool(name="ytp", bufs=2, space="PSUM") as ppy:
            pools = (pS, p8, p4, p2t, p1t, prc, pbc, ppS, ppd, ppy)
            for g in range(N_KV_HEAD):
                kSw = pks.tile([128, T], F16, name="kSw", tag="kSw")
                nc.sync.dma_start(kSw[0:64, :], kT_res[64:128, g, :])
                nc.sync.dma_start(kSw[64:128, :], kT_res[0:64, g, :])
                kA = pkr.tile([128, T], F16, name="kA", tag="kA")
                nc.vector.tensor_mul(kA[:], kT_res[:, g, :], c2ks[:])
                nc.vector.tensor_mul(kSw[:], kSw[:], s2ks[:])
                kR = pkr.tile([128, T], F16, name="kR", tag="kR")
                nc.vector.tensor_add(kR[:], kA[:], kSw[:])
                for hh in range(4):
                    h = g * 4 + hh
                    qTt = pqh.tile([128, TQ], F16, name="qTt", tag="qTt")
                    nc.sync.dma_start(qTt[:], r3(qTs)[h])
                    qSw = pqh.tile([128, TQ], F16, name="qSw", tag="qSw")
                    nc.sync.dma_start(qSw[0:64, :], r3(qTs)[h, 64:128, :])
                    nc.sync.dma_start(qSw[64:128, :], r3(qTs)[h, 0:64, :])
                    qA = pqh.tile([128, TQ], F16, name="qA", tag="qA")
                    nc.vector.tensor_mul(qA[:], qTt[:], c2qs[:])
                    nc.vector.tensor_mul(qSw[:], qSw[:], s2qs[:])
                    qR = pqh.tile([128, TQ], F16, name="qR", tag="qR")
                    nc.vector.tensor_add(qR[:], qA[:], qSw[:])
                    for qc in range(2):
                        p2_block(tc, cs, pools, g, hh, qc, qR, kR)

    def p3(tc, cs):
        """output projection out = yT.T @ wo."""
        yT_res = st["yT"]
        with tc.tile_pool(name="woc", bufs=2) as pwo, \
             tc.tile_pool(name="ev3", bufs=4) as pe3, \
             tc.tile_pool(name="pp3", bufs=4, space="PSUM") as pp3:
            for co in range(4):
                woc = pwo.tile([128, 16, 512], F16, name="woc", tag="woc")
                nc.sync.dma_start(woc[:], rp(wo)[:, :, co * 512:(co + 1) * 512])
                for qt in range(8):
                    ps = pp3.tile([128, 512], F32, name="pso", tag="ps")
                    for yc in range(16):
                        nc.tensor.matmul(
                            ps[:], yT_res[:, yc, qt * 128:(qt + 1) * 128],
                            woc[:, yc, :],
                            start=(yc == 0), stop=(yc == 15),
                        )
                    osb = pe3.tile([128, 512], F32, name="osb", tag="osb")
                    nc.scalar.activation(osb[:], ps[:], AF.Copy)
                    nc.sync.dma_start(
                        out[qt * 128:(qt + 1) * 128, co * 512:(co + 1) * 512],
                        osb[:],
                    )

    with tile.TileContext(nc, trace_sim=trace_sim) as tc:
        with tc.tile_pool(name="const", bufs=1) as cpool:
            cs = {}
            for nm, src in (("ones", ones_d), ("onesq", onesq_d),
                            ("onesk", onesk_d)):
                t = cpool.tile([128, 1], F16, name=nm + "_t")
                nc.sync.dma_start(t[:], src)
                cs[nm] = t
            cs["eps"] = cpool.tile([1, 1], F32, name="eps_t")
            nc.sync.dma_start(cs["eps"][:], eps_d)
            cs["qnw"] = cpool.tile([128, 16], F32, name="qnw_t")
            nc.sync.dma_start(cs["qnw"][:], qnw)
            cs["knw"] = cpool.tile([128, 4], F32, name="knw_t")
            nc.sync.dma_start(cs["knw"][:], knw)

            for rep in range(reps):
                with tc.tile_pool(name="resid", bufs=1) as pres, \
                     tc.tile_pool(name="tabs", bufs=1) as ptab:
                    st.clear()
                    st["pres"], st["ptab"] = pres, ptab
                    p1a(tc, cs)
                    p1b(tc, cs)
                    p2(tc, cs)
                    p3(tc, cs)

    nc.compile()
    return nc


def _make_in_maps(inputs):
    F16 = np.float16
    x = np.asarray(inputs["x"], np.float32)
    cos = np.asarray(inputs["cos"], np.float32)
    sin = np.asarray(inputs["sin"], np.float32)
    wq = np.ascontiguousarray(np.asarray(inputs["wq"], np.float32).astype(F16))
    wk = np.ascontiguousarray(np.asarray(inputs["wk"], np.float32).astype(F16))
    wv = np.ascontiguousarray(np.asarray(inputs["wv"], np.float32).astype(F16))
    wo = np.ascontiguousarray(np.asarray(inputs["wo"], np.float32).astype(F16))
    qnw = np.ascontiguousarray(
        np.asarray(inputs["q_norm_w"], np.float32).reshape(16, 128).T
    )
    knw = np.ascontiguousarray(
        np.asarray(inputs["k_norm_w"], np.float32).reshape(4, 128).T
    )

    cf = cos[0, :, 0, :].T  # (64, T)
    sf = sin[0, :, 0, :].T
    c2k = np.concatenate([cf, cf], 0)  # (128, T)
    s2k = np.concatenate([sf, -sf], 0)
    scale = 1.0 / np.sqrt(np.float32(HEAD_DIM))
    c2k16 = np.ascontiguousarray(c2k.astype(F16))
    s2k16 = np.ascontiguousarray(s2k.astype(F16))

    in_maps = []
    for c in range(N_CORES):
        b, r0 = c // 2, (c % 2) * TQ
        xT = np.ascontiguousarray(x[b].T.astype(F16))
        in_maps.append({
            "xT": xT,
            "xTq": np.ascontiguousarray(xT[:, r0:r0 + TQ]),
            "wq": wq, "wk": wk, "wv": wv, "wo": wo,
            "c2q": np.ascontiguousarray((c2k[:, r0:r0 + TQ] * scale).astype(F16)),
            "s2q": np.ascontiguousarray((s2k[:, r0:r0 + TQ] * scale).astype(F16)),
            "c2k": c2k16, "s2k": s2k16,
            "qnw": qnw, "knw": knw,
        })
    return in_maps


def run(inputs, **spmd_kwargs):
    from concourse import bass_utils

    if "nc" not in _CACHE:
        _CACHE["nc"] = _build_nc()
    nc = _CACHE["nc"]
    res = bass_utils.run_bass_kernel_spmd(
        nc, _make_in_maps(inputs), core_ids=list(range(N_CORES)), **spmd_kwargs
    )
    out = np.empty((B, T, C), np.float32)
    for c in range(N_CORES):
        b, r0 = c // 2, (c % 2) * TQ
        out[b, r0:r0 + TQ, :] = res.results[c]["out"]
    return out, res


def kernel(**inputs):
    out, _ = run(inputs)
    return out


# revision 20
# speedup vs baseline: 1.1919x; 1.0380x over previous
"""Bidirectional GQA attention block (B=4,T=2048,C=2048,H=16,KVH=4) on 8 TRN2 cores.

Sharding: data-parallel over (batch, seq-half): core c handles batch b=c//2,
query tokens [r0, r0+1024) with r0=(c%2)*1024.  k/v are computed for the full
batch sequence on each core (2x duplicated k/v-proj work, ~8% overhead) so no
cross-core communication is needed; the final output is a pure concatenation.

v2 pipeline (everything staged in fp16; all matmuls fp16 at full PE rate;
PSUM accumulation fp32):
  P1a: q^T = wq^T x^T channel-major -> qTs (DRAM, fp16).  Sum-of-squares for
       RMSNorm via (1/C)-ones matmul; rs = 1/sqrt(mean+eps) (ACT sqrt + DVE
       recip), folded into per-token rope tables (q tables pre-scaled by
       1/sqrt(head_dim) on host).
  P1b: k^T and v projections written DIRECTLY into SBUF-resident tiles
       (no DRAM round trip).  Rope k-tables scaled per 512-token chunk.
  P2:  per kv-group g: kR = kT*c2k + kTswap*s2k (swap via SBUF->SBUF DMA);
       per head: qR likewise (q re-read from DRAM).  logits^T = kR_chunk qR
       per 128-key chunk, exp on ACT -> S fp16.  Softmax denominator via a
       4-level DVE pairwise tree (fp16, 2x mode) + ONE 512-row ones-matmul
       per block (16x less PE than the naive ones-matmul).  den rows for the
       8 blocks of a group batch into one [8,512] PSUM tile -> ONE DVE
       reciprocal per group.  y^T = v S accumulated in PSUM, staged to SBUF
       (ACT), divided by den (DVE) into the SBUF-resident yT tile.
  P3:  out = yT.T wo with PSUM accumulation over the 16 head-chunks.
"""
import sys
import os

sys.path.insert(0, "/opt/trn_rl_repo")

import numpy as np

B, T, C = 4, 2048, 2048
N_HEAD, N_KV_HEAD = 16, 4
HEAD_DIM = C // N_HEAD  # 128
KV_DIM = N_KV_HEAD * HEAD_DIM  # 512
EPS = 1e-5
TQ = 1024  # query tokens per core
N_CORES = 8

_CACHE = {}


def _build_nc(reps=1, trace_sim=False):
    import concourse.bass as bass
    import concourse.mybir as mybir
    import concourse.tile as tile
    from concourse import bacc

    F32 = mybir.dt.float32
    F16 = mybir.dt.float16
    AF = mybir.ActivationFunctionType

    nc = bacc.Bacc("TRN2", target_bir_lowering=False, debug=False)

    def ein(name, shape, dt=F16):
        return nc.dram_tensor(name, shape, dt, kind="ExternalInput").ap()

    xT = ein("xT", [C, T])          # x[b].T  (c_in, tok) fp16
    xTq = ein("xTq", [C, TQ])       # x[b].T[:, r0:r0+TQ] fp16
    wq = ein("wq", [C, C])
    wk = ein("wk", [C, KV_DIM])
    wv = ein("wv", [C, KV_DIM])
    wo = ein("wo", [C, C])
    c2q = ein("c2q", [128, TQ])     # [cos;cos] / sqrt(HEAD_DIM), q token slice
    s2q = ein("s2q", [128, TQ])     # [sin;-sin] / sqrt(HEAD_DIM)
    c2k = ein("c2k", [128, T])      # [cos;cos] (unscaled)
    s2k = ein("s2k", [128, T])
    qnw = ein("qnw", [128, 16], F32)  # q_norm_w.reshape(16,128).T
    knw = ein("knw", [128, 4], F32)
    out = nc.dram_tensor("out", [TQ, C], F32, kind="ExternalOutput").ap()

    ones_d = nc.inline_tensor(np.ones((128, 1), np.float16), name="onesc").ap()
    onesq_d = nc.inline_tensor(
        np.full((128, 1), 1.0 / C, np.float16), name="onesqc"
    ).ap()
    onesk_d = nc.inline_tensor(
        np.full((128, 1), 1.0 / KV_DIM, np.float16), name="oneskc"
    ).ap()
    eps_d = nc.inline_tensor(np.full((1, 1), EPS, np.float32), name="epsc").ap()

    # DRAM scratch: only q^T is staged (k/v/y live in SBUF)
    qTs = nc.dram_tensor("qTs", [C, TQ], F16).ap()

    def r3(ap, p=128):
        # (c*p, n) -> (c, p, n)
        return ap.rearrange("(c p) n -> c p n", p=p)

    def rp(ap, p=128):
        # (c*p, n) -> (p, c, n)
        return ap.rearrange("(c p) n -> p c n", p=p)

    # state shared between phases of one rep
    st = {}

    def p1a(tc, cs):
        """q^T projection + rmsnorm stats; loads rope tables; allocates
        SBUF-resident k/v/y tiles."""
        with tc.tile_pool(name="p1q", bufs=1) as p1, \
             tc.tile_pool(name="wqlp", bufs=2) as pw, \
             tc.tile_pool(name="ev1", bufs=3) as pe, \
             tc.tile_pool(name="tmp1", bufs=3) as pt, \
             tc.tile_pool(name="rsp", bufs=2) as prs, \
             tc.tile_pool(name="pp1", bufs=4, space="PSUM") as pp, \
             tc.tile_pool(name="ssqp", bufs=1, space="PSUM") as pps:
            xqs = []
            for tq in range(2):
                xq = p1.tile([128, 16, 512], F16, name=f"xq{tq}", tag=f"xq{tq}")
                nc.sync.dma_start(xq[:], rp(xTq)[:, :, tq * 512:(tq + 1) * 512])
                xqs.append(xq)
            # rope tables (scaled at end of P1a / in P1b)
            for nm, src in (("c2qs", c2q), ("s2qs", s2q),
                            ("c2ks", c2k), ("s2ks", s2k)):
                t = st["ptab"].tile([128, src.shape[-1]], F16, name=nm)
                nc.sync.dma_start(t[:], src)
                st[nm] = t
            # prefetch P1b weights so k/v proj starts without a DMA stall
            wkt = st["pwkv"].tile([128, 16, KV_DIM], F16, name="wkt")
            nc.sync.dma_start(wkt[:], rp(wk))
            st["wkt"] = wkt
            wvt = st["pwkv"].tile([128, 16, KV_DIM], F16, name="wvt")
            nc.sync.dma_start(wvt[:], rp(wv))
            st["wvt"] = wvt
            # SBUF-resident k/v/y
            st["kT"] = st["pres"].tile([128, 4, T], F16, name="kT_res")
            st["v"] = st["pres"].tile([128, 16, KV_DIM], F16, name="v_res")
            st["yT"] = st["pres"].tile([128, 16, TQ], F16, name="yT_res")
            # wo chunk 0, loaded during P2 (g=3) for a stall-free P3 start
            st["woc0"] = st["ptab"].tile([128, 16, 512], F16, name="woc0")

            ssq_ps = [
                pps.tile([1, 512], F32, name=f"ssqq{tq}", tag=f"ssqq{tq}")
                for tq in range(2)
            ]
            for cout in range(16):
                wql = pw.tile([128, 16, 128], F16, name="wql", tag="wql")
                nc.sync.dma_start(
                    wql[:], rp(wq)[:, :, cout * 128:(cout + 1) * 128]
                )
                for tq in range(2):
                    ps = pp.tile([128, 512], F32, name="psq", tag="ps")
                    for kc in range(16):
                        nc.tensor.matmul(
                            ps[:], wql[:, kc, :], xqs[tq][:, kc, :],
                            start=(kc == 0), stop=(kc == 15),
                        )
                    qsb = pe.tile([128, 512], F16, name="qsb", tag="qsb")
                    nc.scalar.activation(
                        qsb[:], ps[:], AF.Copy, scale=cs["qnw"][:, cout:cout + 1]
                    )
                    # stores ride the gpsimd queue so they never block loads
                    nc.gpsimd.dma_start(
                        r3(qTs)[cout, :, tq * 512:(tq + 1) * 512], qsb[:]
                    )
                    sq = pt.tile([128, 512], F16, name="sqq", tag="sq")
                    nc.scalar.activation(sq[:], ps[:], AF.Square)
                    nc.tensor.matmul(
                        ssq_ps[tq][:], cs["onesq"][:], sq[:],
                        start=(cout == 0), stop=(cout == 15),
                    )
            for tq in range(2):
                sl = slice(tq * 512, (tq + 1) * 512)
                sd = prs.tile([1, 512], F32, name="sdq", tag="sdq")
                nc.scalar.activation(sd[:], ssq_ps[tq][:], AF.Sqrt,
                                     bias=cs["eps"][:])
                rs = prs.tile([1, 512], F32, name="rsq", tag="rsq")
                nc.vector.reciprocal(rs[:], sd[:])
                bcq = prs.tile([128, 512], F32, name="bcq", tag="bcq")
                nc.gpsimd.partition_broadcast(bcq[:], rs[:])
                nc.vector.tensor_mul(st["c2qs"][:, sl], st["c2qs"][:, sl], bcq[:])
                nc.vector.tensor_mul(st["s2qs"][:, sl], st["s2qs"][:, sl], bcq[:])

    def p1b(tc, cs):
        """k^T and v projections into SBUF-resident tiles + k-table scaling."""
        kT_res, v_res = st["kT"], st["v"]
        wkt, wvt = st["wkt"], st["wvt"]
        with tc.tile_pool(name="xsp", bufs=2) as pxs, \
             tc.tile_pool(name="tmp2", bufs=3) as pt, \
             tc.tile_pool(name="rsk", bufs=2) as prs, \
             tc.tile_pool(name="pp2", bufs=4, space="PSUM") as pp, \
             tc.tile_pool(name="ssqk", bufs=2, space="PSUM") as pps:
            for tk in range(4):
                tsl = slice(tk * 512, (tk + 1) * 512)
                xs = pxs.tile([128, 16, 512], F16, name="xsc", tag="xsc")
                nc.sync.dma_start(xs[:], rp(xT)[:, :, tsl])
                ssqk_ps = pps.tile([1, 512], F32, name="ssqk", tag="ssqk")
                for co in range(4):
                    ps = pp.tile([128, 512], F32, name="psk", tag="ps")
                    for kc in range(16):
                        nc.tensor.matmul(
                            ps[:], wkt[:, kc, co * 128:(co + 1) * 128], xs[:, kc, :],
                            start=(kc == 0), stop=(kc == 15),
                        )
                    nc.scalar.activation(
                        kT_res[:, co, tsl], ps[:], AF.Copy,
                        scale=cs["knw"][:, co:co + 1]
                    )
                    sq = pt.tile([128, 512], F16, name="sqk", tag="sq")
                    nc.scalar.activation(sq[:], ps[:], AF.Square)
                    nc.tensor.matmul(
                        ssqk_ps[:], cs["onesk"][:], sq[:],
                        start=(co == 0), stop=(co == 3),
                    )
                sd = prs.tile([1, 512], F32, name="sdk", tag="sdk")
                nc.scalar.activation(sd[:], ssqk_ps[:], AF.Sqrt, bias=cs["eps"][:])
                rs = prs.tile([1, 512], F32, name="rsk", tag="rsk")
                nc.vector.reciprocal(rs[:], sd[:])
                bck = prs.tile([128, 512], F32, name="bck", tag="bck")
                nc.gpsimd.partition_broadcast(bck[:], rs[:])
                nc.vector.tensor_mul(st["c2ks"][:, tsl], st["c2ks"][:, tsl], bck[:])
                nc.vector.tensor_mul(st["s2ks"][:, tsl], st["s2ks"][:, tsl], bck[:])
                for vt in range(4):
                    ps = pp.tile([128, 512], F32, name="psv", tag="ps")
                    for kc in range(16):
                        nc.tensor.matmul(
                            ps[:], xs[:, kc, vt * 128:(vt + 1) * 128], wvt[:, kc, :],
                            start=(kc == 0), stop=(kc == 15),
                        )
                    nc.scalar.activation(v_res[:, tk * 4 + vt, :], ps[:], AF.Copy)

    def p2_head(tc, cs, pools, g, hh, qR, kR):
        """One head: both q-chunks' S matmuls + exps first (one long PE
        stretch; exp of chunk 0 overlaps S matmuls of chunk 1), then the
        DVE den trees, then yt/den matmuls and the division.  The den
        matmul sits AFTER yt so the in-order PE never waits on the tree."""
        pS, p8, p4, p2t, p1t, prc, pbc, ppS, ppd, ppy = pools
        h = g * 4 + hh
        g128 = slice(g * 128, (g + 1) * 128)
        S_sbs, t1s = [], []
        for qc in range(2):
            qsl = slice(qc * 512, (qc + 1) * 512)
            S_sb = pS.tile([128, 16, 512], F16, name="S_sb", tag="S")
            for j in range(8):
                sps = ppS.tile([128, 2, 512], F32, name="sps", tag="sps")
                for i in range(2):
                    kc = 2 * j + i
                    nc.tensor.matmul(
                        sps[:, i, :], kR[:, kc * 128:(kc + 1) * 128], qR[:, qsl],
                        start=True, stop=True,
                    )
                nc.scalar.activation(S_sb[:, 2 * j:2 * j + 2, :], sps[:], AF.Exp)
            S_sbs.append(S_sb)
        for qc in range(2):
            # denominator: 4-level pairwise tree on DVE (fp16, 2x mode)
            S_sb = S_sbs[qc]
            t8 = p8.tile([128, 8, 512], F16, name="t8", tag="t8")
            nc.vector.tensor_add(t8[:], S_sb[:, 0:8, :], S_sb[:, 8:16, :])
            t4 = p4.tile([128, 4, 512], F16, name="t4", tag="t4")
            nc.vector.tensor_add(t4[:], t8[:, 0:4, :], t8[:, 4:8, :])
            t2 = p2t.tile([128, 2, 512], F16, name="t2", tag="t2")
            nc.vector.tensor_add(t2[:], t4[:, 0:2, :], t4[:, 2:4, :])
            t1 = p1t.tile([128, 512], F16, name="t1", tag="t1")
            nc.vector.tensor_add(t1[:], t2[:, 0, :], t2[:, 1, :])
            t1s.append(t1)
        for qc in range(2):
            qsl = slice(qc * 512, (qc + 1) * 512)
            yt_ps = ppy.tile([128, 512], F32, name="ytp", tag="ytp")
            for kc in range(16):
                nc.tensor.matmul(
                    yt_ps[:], st["v"][:, kc, g128], S_sbs[qc][:, kc, :],
                    start=(kc == 0), stop=(kc == 15),
                )
            den_ps = ppd.tile([1, 512], F32, name="den", tag="den")
            nc.tensor.matmul(den_ps[:], cs["ones"][:], t1s[qc][:],
                             start=True, stop=True)
            rcp = prc.tile([1, 512], F32, name="rcp", tag="rcp")
            nc.vector.reciprocal(rcp[:], den_ps[:])
            bcr = pbc.tile([128, 512], F32, name="bcr", tag="bcr")
            nc.gpsimd.partition_broadcast(bcr[:], rcp[:])
            nc.vector.tensor_mul(st["yT"][:, h, qsl], yt_ps[:], bcr[:])

    def p2(tc, cs):
        """attention over 4 kv-groups x 4 heads x 2 q-chunks."""
        kT_res, yT_res = st["kT"], st["yT"]
        c2qs, s2qs, c2ks, s2ks = st["c2qs"], st["s2qs"], st["c2ks"], st["s2ks"]
        with tc.tile_pool(name="ksw", bufs=2) as pks, \
             tc.tile_pool(name="krp", bufs=2) as pkr, \
             tc.tile_pool(name="qh", bufs=2) as pqh, \
             tc.tile_pool(name="Sp", bufs=2) as pS, \
             tc.tile_pool(name="tr8", bufs=1) as p8, \
             tc.tile_pool(name="tr4", bufs=1) as p4, \
             tc.tile_pool(name="tr2", bufs=1) as p2t, \
             tc.tile_pool(name="tr1", bufs=2) as p1t, \
             tc.tile_pool(name="rcb", bufs=2) as prc, \
             tc.tile_pool(name="bcb", bufs=2) as pbc, \
             tc.tile_pool(name="sps", bufs=2, space="PSUM") as ppS, \
             tc.tile_pool(name="dnp", bufs=2, space="PSUM") as ppd, \
             tc.tile_pool(name="ytp", bufs=2, space="PSUM") as ppy:
            pools = (pS, p8, p4, p2t, p1t, prc, pbc, ppS, ppd, ppy)
            for g in range(N_KV_HEAD):
                kSw = pks.tile([128, T], F16, name="kSw", tag="kSw")
                nc.sync.dma_start(kSw[0:64, :], kT_res[64:128, g, :])
                nc.sync.dma_start(kSw[64:128, :], kT_res[0:64, g, :])
                kA = pkr.tile([128, T], F16, name="kA", tag="kA", bufs=1)
                nc.vector.tensor_mul(kA[:], kT_res[:, g, :], c2ks[:])
                nc.vector.tensor_mul(kSw[:], kSw[:], s2ks[:])
                kR = pkr.tile([128, T], F16, name="kR", tag="kR")
                nc.vector.tensor_add(kR[:], kA[:], kSw[:])
                if g == 3:
                    # prefetch wo chunk 0 during the last attention group
                    nc.sync.dma_start(st["woc0"][:], rp(wo)[:, :, 0:512])
                for hh in range(4):
                    h = g * 4 + hh
                    qTt = pqh.tile([128, TQ], F16, name="qTt", tag="qTt")
                    nc.sync.dma_start(qTt[:], r3(qTs)[h])
                    qSw = pqh.tile([128, TQ], F16, name="qSw", tag="qSw")
                    nc.sync.dma_start(qSw[0:64, :], r3(qTs)[h, 64:128, :])
                    nc.sync.dma_start(qSw[64:128, :], r3(qTs)[h, 0:64, :])
                    qA = pqh.tile([128, TQ], F16, name="qA", tag="qA", bufs=1)
                    nc.vector.tensor_mul(qA[:], qTt[:], c2qs[:])
                    nc.vector.tensor_mul(qSw[:], qSw[:], s2qs[:])
                    qR = pqh.tile([128, TQ], F16, name="qR", tag="qR")
                    nc.vector.tensor_add(qR[:], qA[:], qSw[:])
                    p2_head(tc, cs, pools, g, hh, qR, kR)

    def p3(tc, cs):
        """output projection out = yT.T @ wo."""
        yT_res = st["yT"]
        with tc.tile_pool(name="woc", bufs=2) as pwo, \
             tc.tile_pool(name="ev3", bufs=4) as pe3, \
             tc.tile_pool(name="pp3", bufs=4, space="PSUM") as pp3:
            for co in range(4):
                if co == 0:
                    woc = st["woc0"]  # prefetched during P2
                else:
                    woc = pwo.tile([128, 16, 512], F16, name="woc", tag="woc")
                    nc.sync.dma_start(
                        woc[:], rp(wo)[:, :, co * 512:(co + 1) * 512]
                    )
                for qt in range(8):
                    ps = pp3.tile([128, 512], F32, name="pso", tag="ps")
                    for yc in range(16):
                        nc.tensor.matmul(
                            ps[:], yT_res[:, yc, qt * 128:(qt + 1) * 128],
                            woc[:, yc, :],
                            start=(yc == 0), stop=(yc == 15),
                        )
                    osb = pe3.tile([128, 512], F32, name="osb", tag="osb")
                    nc.scalar.activation(osb[:], ps[:], AF.Copy)
                    nc.gpsimd.dma_start(
                        out[qt * 128:(qt + 1) * 128, co * 512:(co + 1) * 512],
                        osb[:],
                    )

    with tile.TileContext(nc, trace_sim=trace_sim) as tc:
        with tc.tile_pool(name="const", bufs=1) as cpool:
            cs = {}
            for nm, src in (("ones", ones_d), ("onesq", onesq_d),
                            ("onesk", onesk_d)):
                t = cpool.tile([128, 1], F16, name=nm + "_t")
                nc.sync.dma_start(t[:], src)
                cs[nm] = t
            cs["eps"] = cpool.tile([1, 1], F32, name="eps_t")
            nc.sync.dma_start(cs["eps"][:], eps_d)
            cs["qnw"] = cpool.tile([128, 16], F32, name="qnw_t")
            nc.sync.dma_start(cs["qnw"][:], qnw)
            cs["knw"] = cpool.tile([128, 4], F32, name="knw_t")
            nc.sync.dma_start(cs["knw"][:], knw)

            for rep in range(reps):
                with tc.tile_pool(name="resid", bufs=1) as pres, \
                     tc.tile_pool(name="tabs", bufs=1) as ptab:
                    st.clear()
                    st["pres"], st["ptab"] = pres, ptab
                    with tc.tile_pool(name="wkv", bufs=1) as pwkv:
                        st["pwkv"] = pwkv
                        p1a(tc, cs)
                        p1b(tc, cs)
                    p2(tc, cs)
                    p3(tc, cs)

    nc.compile()
    return nc


def _make_in_maps(inputs):
    F16 = np.float16
    x = np.asarray(inputs["x"], np.float32)
    cos = np.asarray(inputs["cos"], np.float32)
    sin = np.asarray(inputs["sin"], np.float32)
    wq = np.ascontiguousarray(np.asarray(inputs["wq"], np.float32).astype(F16))
    wk = np.ascontiguousarray(np.asarray(inputs["wk"], np.float32).astype(F16))
    wv = np.ascontiguousarray(np.asarray(inputs["wv"], np.float32).astype(F16))
    wo = np.ascontiguousarray(np.asarray(inputs["wo"], np.float32).astype(F16))
    qnw = np.ascontiguousarray(
        np.asarray(inputs["q_norm_w"], np.float32).reshape(16, 128).T
    )
    knw = np.ascontiguousarray(
        np.asarray(inputs["k_norm_w"], np.float32).reshape(4, 128).T
    )

    cf = cos[0, :, 0, :].T  # (64, T)
    sf = sin[0, :, 0, :].T
    c2k = np.concatenate([cf, cf], 0)  # (128, T)
    s2k = np.concatenate([sf, -sf], 0)
    scale = 1.0 / np.sqrt(np.float32(HEAD_DIM))
    c2k16 = np.ascontiguousarray(c2k.astype(F16))
    s2k16 = np.ascontiguousarray(s2k.astype(F16))

    in_maps = []
    for c in range(N_CORES):
        b, r0 = c // 2, (c % 2) * TQ
        xT = np.ascontiguousarray(x[b].T.astype(F16))
        in_maps.append({
            "xT": xT,
            "xTq": np.ascontiguousarray(xT[:, r0:r0 + TQ]),
            "wq": wq, "wk": wk, "wv": wv, "wo": wo,
            "c2q": np.ascontiguousarray((c2k[:, r0:r0 + TQ] * scale).astype(F16)),
            "s2q": np.ascontiguousarray((s2k[:, r0:r0 + TQ] * scale).astype(F16)),
            "c2k": c2k16, "s2k": s2k16,
            "qnw": qnw, "knw": knw,
        })
    return in_maps


def run(inputs, **spmd_kwargs):
    from concourse import bass_utils

    if "nc" not in _CACHE:
        _CACHE["nc"] = _build_nc()
    nc = _CACHE["nc"]
    res = bass_utils.run_bass_kernel_spmd(
        nc, _make_in_maps(inputs), core_ids=list(range(N_CORES)), **spmd_kwargs
    )
    out = np.empty((B, T, C), np.float32)
    for c in range(N_CORES):
        b, r0 = c // 2, (c % 2) * TQ
        out[b, r0:r0 + TQ, :] = res.results[c]["out"]
    return out, res


def kernel(**inputs):
    out, _ = run(inputs)
    return out
